# revision 13
# baseline (speedup 1.0000x reference)
"""CIN (xDeepFM CompressedInteractionNetwork) forward on 8 TRN2 NeuronCores.

Strategy (pure data parallelism, hardcoded from the problem spec):
  - batch 4096 -> 512 per core; 64 "tiles" of 8 batch elements; matmul
    free dim = 512 columns = (8 batch x 64 embed).
  - layer l: out[o, col] = relu( sum_c W[o,c] * z[c, col] + b[o] ) with
    z[f*Hin+j, col] = x0[f, col] * h[j, col], materialized on the vector
    engine as bf16 tensor_tensor multiplies against a partition-broadcast
    copy of the x rows (XR).
  - host pre-layout xb as [NT, F, NB*E]: each tile's [32, 512] x-slice is
    contiguous, so XR broadcast DMAs move 8KB-contiguous per destination
    partition (512 packets/tile instead of ~4K 1KB packets).
  - z is generated in [128, 8, 512] chunks so the PE starts consuming
    after the first chunk instead of waiting for a full layer's z.
  - software pipeline across tiles: tensor-queue order per iteration is
    L0(t), L2(t-1), L1(t) - the previous tile's layer-2 matmuls fill the
    bubble while h1(t) -> z1(t) is produced, keeping the PE (and its
    2.4GHz p-state) continuously busy.
  - matmuls bf16 (full PE rate), fp32 PSUM, all 8 PSUM banks.
  - ScalarE applies bias+relu straight out of PSUM; pooling (sum over
    embed) via vector tensor_reduce into bf16 accumulators; final FC on
    host (tiny).

bf16 end-to-end error vs fp32 reference measured at ~4e-3 L2 relative.
"""

import sys

sys.path.insert(0, "/opt/trn_rl_repo")

import numpy as np
import ml_dtypes
from contextlib import ExitStack

N_CORES = 8
B = 4096
F = 32
E = 64
BC = B // N_CORES  # 512 batch elements per core
NB = 8             # batch elements per tile
COLS = NB * E      # 512 matmul columns per tile
NT = BC // NB      # 64 tiles per core
O = 256            # conv out channels per layer
H = 128            # h channels (split_half) for layers 1,2
NCH = 4            # z chunks per layer (8 k-slices each)

_CACHE = {}


def _build(n_tiles=NT):
    import concourse.bass as bass  # noqa: F401
    import concourse.mybir as mybir
    import concourse.tile as tile
    from concourse import bacc

    dt = mybir.dt
    AF = mybir.ActivationFunctionType
    ALU = mybir.AluOpType
    AX = mybir.AxisListType

    nc = bacc.Bacc("TRN2", target_bir_lowering=False, debug=False,
                   num_devices=N_CORES)

    xb = nc.declare_dram_parameter("xb", [n_tiles, F, COLS], dt.bfloat16,
                                   isOutput=False)
    w0t = nc.declare_dram_parameter("w0t", [F * F, O], dt.bfloat16, isOutput=False)
    w1t = nc.declare_dram_parameter("w1t", [F * H, O], dt.bfloat16, isOutput=False)
    w2t = nc.declare_dram_parameter("w2t", [F * H, O], dt.bfloat16, isOutput=False)
    b0 = nc.declare_dram_parameter("b0", [O], dt.float32, isOutput=False)
    b1 = nc.declare_dram_parameter("b1", [O], dt.float32, isOutput=False)
    b2 = nc.declare_dram_parameter("b2", [O], dt.float32, isOutput=False)
    pout = nc.declare_dram_parameter("pout", [4, 128, n_tiles * NB],
                                     dt.bfloat16, isOutput=True)

    with ExitStack() as ctx:
        ctx.enter_context(nc.allow_low_precision(
            reason="bf16 pooled sums validated at ~4e-3 rel err vs fp32 ref"))
        tc = ctx.enter_context(tile.TileContext(nc))
        const = ctx.enter_context(tc.tile_pool(name="const", bufs=1))

        # ---- persistent weights / biases ----
        # lw0 + biases first (needed immediately); lw1/lw2 DMAs are emitted
        # after tile 0's input DMAs below (they are needed ~25/60us in).
        lw0 = const.tile([128, 8, O], dt.bfloat16)       # w0t chunked [c=128g+p]
        nc.sync.dma_start(lw0[:], w0t.ap().rearrange("(g p) o -> p g o", p=128))
        lw1 = const.tile([128, 32, O], dt.bfloat16)
        lw2 = const.tile([128, 32, O], dt.bfloat16)

        bias0 = const.tile([128, 2], dt.float32)
        nc.sync.dma_start(bias0[:], b0.ap().rearrange("(m p) -> p m", p=128))
        bias1 = const.tile([128, 2], dt.float32)
        nc.sync.dma_start(bias1[:], b1.ap().rearrange("(m p) -> p m", p=128))
        bias2 = const.tile([128, 2], dt.float32)
        nc.sync.dma_start(bias2[:], b2.ap().rearrange("(m p) -> p m", p=128))

        # pooled accumulators [o_chunk 128, batch 512]
        P0 = const.tile([128, n_tiles * NB], dt.bfloat16)
        P1 = const.tile([128, n_tiles * NB], dt.bfloat16)
        P2a = const.tile([128, n_tiles * NB], dt.bfloat16)
        P2b = const.tile([128, n_tiles * NB], dt.bfloat16)

        # ---- per-tile pools ----
        xr_pool = ctx.enter_context(tc.tile_pool(name="xr", bufs=2))
        xr0_pool = ctx.enter_context(tc.tile_pool(name="xr0", bufs=2))
        x0r_pool = ctx.enter_context(tc.tile_pool(name="x0r", bufs=2))
        z_pool = ctx.enter_context(tc.tile_pool(name="z", bufs=9))
        h_pool = ctx.enter_context(tc.tile_pool(name="h", bufs=3))
        r_pool = ctx.enter_context(tc.tile_pool(name="r", bufs=3))
        psum_pool = ctx.enter_context(tc.tile_pool(name="ps", bufs=8, space="PSUM"))

        def emit_dma(t):
            """Prefetch tile t's broadcast inputs. Returns (xr, xr0, x0rep).

            Layer-0 operands (xr0, x0rep) are emitted before the big XR
            broadcast so tile 0's layer 0 can start ASAP at kernel start.
            """
            xsl = xb.ap()[t]  # [F, COLS], fully contiguous
            # XR0: [128, 8, COLS]; value[p, g, col] = xsl[4g + (p>>5), col]
            xr0 = xr0_pool.tile([128, 8, COLS], dt.bfloat16, name="xr0", tag="x0")
            for fh in range(4):
                src = xsl.rearrange("(g fh) c -> fh g c", fh=4)[fh] \
                    .unsqueeze(0).broadcast_to([32, 8, COLS])
                nc.sync.dma_start(xr0[fh * 32:(fh + 1) * 32], src)
            # x0rep: [128, COLS]; value[p, col] = xsl[p & 31, col]
            x0rep = x0r_pool.tile([128, COLS], dt.bfloat16, name="x0r", tag="x0r")
            for k in range(4):
                nc.sync.dma_start(x0rep[k * 32:(k + 1) * 32], xsl)
            # XR: [128, 32, COLS]; value[p, f, col] = xsl[f, col].
            # 8 DMAs of 4KB-contiguous per partition, spread over queues.
            xr_t = xr_pool.tile([128, F, COLS], dt.bfloat16, name="xr", tag="xr")
            for k in range(8):
                src = xsl[k * 4:(k + 1) * 4].rearrange("f c -> (f c)") \
                    .unsqueeze(0).broadcast_to([128, 4 * COLS])
                nc.sync.dma_start(
                    xr_t[:, k * 4:(k + 1) * 4, :].rearrange("p a b -> p (a b)"),
                    src)
            return xr_t, xr0, x0rep

        def z0_gen(tiles):
            xr_t, xr0, x0rep = tiles
            z0 = z_pool.tile([128, 8, COLS], dt.bfloat16, name="z0", tag="z")
            nc.vector.tensor_tensor(
                z0[:], xr0[:],
                x0rep[:].unsqueeze(1).broadcast_to([128, 8, COLS]), ALU.mult)
            return z0

        def reduce_into(P, t, r):
            nc.vector.tensor_reduce(
                P[:, t * NB:(t + 1) * NB],
                r[:].rearrange("p (b e) -> p b e", e=E), AX.X, ALU.add)

        # state carried across loop iterations
        tiles_cur = emit_dma(0)     # tile 0 inputs
        z0_cur = z0_gen(tiles_cur)  # z0 for tile 0
        nc.sync.dma_start(lw1[:], w1t.ap().rearrange("(g p) o -> p g o", p=128))
        nc.sync.dma_start(lw2[:], w2t.ap().rearrange("(g p) o -> p g o", p=128))
        prev = None                 # (t-1, xr(t-1), h2(t-1)) pending layer 2

        for t in range(n_tiles):
            xr_t, xr0_t, x0rep_t = tiles_cur

            # ---- layer 0 (tile t): 16 matmuls from z0 ----
            ps0 = [psum_pool.tile([128, COLS], dt.float32, name="ps0a", tag="ps"),
                   psum_pool.tile([128, COLS], dt.float32, name="ps0b", tag="ps")]
            for m in range(2):
                for g in range(8):
                    nc.tensor.matmul(
                        ps0[m][:], lw0[:, g, m * 128:(m + 1) * 128],
                        z0_cur[:, g, :], start=(g == 0), stop=(g == 7))
            h1 = h_pool.tile([128, COLS], dt.bfloat16, name="h1", tag="h")
            nc.scalar.activation(h1[:], ps0[1][:], AF.Relu, bias=bias0[:, 1:2])
            r0 = r_pool.tile([128, COLS], dt.bfloat16, name="r0", tag="r")
            nc.scalar.activation(r0[:], ps0[0][:], AF.Relu, bias=bias0[:, 0:1])

            # prefetch tile t+1 (DMA waits on xr buffer WAR automatically;
            # emitted after layer 0 so tile t's own transfers win the queues
            # during pipeline fill)
            if t + 1 < n_tiles:
                tiles_nxt = emit_dma(t + 1)

            # ---- layer 2 of tile t-1: z2 chunks + matmuls fill the
            # bubble while h1(t) -> z1(t) is produced ----
            if prev is not None:
                tp, xr_p, h2_p = prev
                ps2 = [psum_pool.tile([128, COLS], dt.float32, name="ps2a", tag="ps"),
                       psum_pool.tile([128, COLS], dt.float32, name="ps2b", tag="ps")]
                for gc in range(NCH):
                    z2c = z_pool.tile([128, 8, COLS], dt.bfloat16,
                                      name=f"z2c{gc}", tag="z")
                    nc.vector.tensor_tensor(
                        z2c[:], xr_p[:, gc * 8:(gc + 1) * 8, :],
                        h2_p[:].unsqueeze(1).broadcast_to([128, 8, COLS]),
                        ALU.mult)
                    for m in range(2):
                        for g in range(8):
                            nc.tensor.matmul(
                                ps2[m][:],
                                lw2[:, gc * 8 + g, m * 128:(m + 1) * 128],
                                z2c[:, g, :],
                                start=(gc == 0 and g == 0),
                                stop=(gc == NCH - 1 and g == 7))
                r2a = r_pool.tile([128, COLS], dt.bfloat16, name="r2a", tag="r")
                nc.scalar.activation(r2a[:], ps2[0][:], AF.Relu, bias=bias2[:, 0:1])
                r2b = r_pool.tile([128, COLS], dt.bfloat16, name="r2b", tag="r")
                nc.scalar.activation(r2b[:], ps2[1][:], AF.Relu, bias=bias2[:, 1:2])
                reduce_into(P2a, tp, r2a)
                reduce_into(P2b, tp, r2b)

            # ---- layer 1 (tile t): z1 chunks + matmuls ----
            ps1 = [psum_pool.tile([128, COLS], dt.float32, name="ps1a", tag="ps"),
                   psum_pool.tile([128, COLS], dt.float32, name="ps1b", tag="ps")]
            for gc in range(NCH):
                z1c = z_pool.tile([128, 8, COLS], dt.bfloat16,
                                  name=f"z1c{gc}", tag="z")
                nc.vector.tensor_tensor(
                    z1c[:], xr_t[:, gc * 8:(gc + 1) * 8, :],
                    h1[:].unsqueeze(1).broadcast_to([128, 8, COLS]),
                    ALU.mult)
                for m in range(2):
                    for g in range(8):
                        nc.tensor.matmul(
                            ps1[m][:],
                            lw1[:, gc * 8 + g, m * 128:(m + 1) * 128],
                            z1c[:, g, :],
                            start=(gc == 0 and g == 0),
                            stop=(gc == NCH - 1 and g == 7))
            h2 = h_pool.tile([128, COLS], dt.bfloat16, name="h2", tag="h")
            nc.scalar.activation(h2[:], ps1[1][:], AF.Relu, bias=bias1[:, 1:2])
            r1 = r_pool.tile([128, COLS], dt.bfloat16, name="r1", tag="r")
            nc.scalar.activation(r1[:], ps1[0][:], AF.Relu, bias=bias1[:, 0:1])
            reduce_into(P0, t, r0)
            reduce_into(P1, t, r1)

            # z0 for tile t+1 (inputs arrive mid-iteration)
            if t + 1 < n_tiles:
                z0_cur = z0_gen(tiles_nxt)
                tiles_cur = tiles_nxt
            prev = (t, xr_t, h2)

        # P0/P1 are complete; ship them while the drain layer-2 runs
        nc.sync.dma_start(pout.ap()[0], P0[:])
        nc.sync.dma_start(pout.ap()[1], P1[:])

        # ---- drain: layer 2 of the last tile ----
        tp, xr_p, h2_p = prev
        ps2 = [psum_pool.tile([128, COLS], dt.float32, name="ps2a", tag="ps"),
               psum_pool.tile([128, COLS], dt.float32, name="ps2b", tag="ps")]
        for gc in range(NCH):
            z2c = z_pool.tile([128, 8, COLS], dt.bfloat16,
                              name=f"z2c{gc}", tag="z")
            nc.vector.tensor_tensor(
                z2c[:], xr_p[:, gc * 8:(gc + 1) * 8, :],
                h2_p[:].unsqueeze(1).broadcast_to([128, 8, COLS]), ALU.mult)
            for m in range(2):
                for g in range(8):
                    nc.tensor.matmul(
                        ps2[m][:], lw2[:, gc * 8 + g, m * 128:(m + 1) * 128],
                        z2c[:, g, :],
                        start=(gc == 0 and g == 0),
                        stop=(gc == NCH - 1 and g == 7))
        r2a = r_pool.tile([128, COLS], dt.bfloat16, name="r2a", tag="r")
        nc.scalar.activation(r2a[:], ps2[0][:], AF.Relu, bias=bias2[:, 0:1])
        r2b = r_pool.tile([128, COLS], dt.bfloat16, name="r2b", tag="r")
        nc.scalar.activation(r2b[:], ps2[1][:], AF.Relu, bias=bias2[:, 1:2])
        reduce_into(P2a, tp, r2a)
        reduce_into(P2b, tp, r2b)

        # ---- ship remaining pooled accumulators; tiny FC happens on host ----
        nc.sync.dma_start(pout.ap()[2], P2a[:])
        nc.sync.dma_start(pout.ap()[3], P2b[:])

    nc.compile()
    return nc


def _prep_inputs(x, w0, b0, w1, b1, w2, b2, fc_w, fc_b):
    bf16 = ml_dtypes.bfloat16
    xb = np.asarray(x, dtype=np.float32).astype(bf16)
    w0t = np.ascontiguousarray(np.asarray(w0, np.float32).T).astype(bf16)
    w1t = np.ascontiguousarray(np.asarray(w1, np.float32).T).astype(bf16)
    w2t = np.ascontiguousarray(np.asarray(w2, np.float32).T).astype(bf16)
    common = {
        "w0t": w0t, "w1t": w1t, "w2t": w2t,
        "b0": np.ascontiguousarray(np.asarray(b0, np.float32)),
        "b1": np.ascontiguousarray(np.asarray(b1, np.float32)),
        "b2": np.ascontiguousarray(np.asarray(b2, np.float32)),
    }
    in_maps = []
    for c in range(N_CORES):
        m = dict(common)
        # [BC, F, E] -> [NT, NB, F, E] -> [NT, F, NB, E] -> [NT, F, NB*E]
        xc = xb[c * BC:(c + 1) * BC].reshape(NT, NB, F, E)
        m["xb"] = np.ascontiguousarray(
            xc.transpose(0, 2, 1, 3)).reshape(NT, F, NB * E)
        in_maps.append(m)
    return in_maps


def kernel(x, w0, b0, w1, b1, w2, b2, fc_w, fc_b, **kw):
    from concourse.bass_utils import run_bass_kernel_spmd

    if "nc" not in _CACHE:
        _CACHE["nc"] = _build()
    nc = _CACHE["nc"]
    in_maps = _prep_inputs(x, w0, b0, w1, b1, w2, b2, fc_w, fc_b)
    res = run_bass_kernel_spmd(nc, in_maps, list(range(N_CORES)))
    fcw = np.asarray(fc_w, np.float32).reshape(4, 128)
    ys = []
    for c in range(N_CORES):
        p = res.results[c]["pout"]  # [4, 128, BC] bf16
        ys.append(np.einsum('cp,cpb->b', fcw, p.astype(np.float32)))
    out = np.concatenate(ys).reshape(B, 1).astype(np.float32)
    out = out + np.asarray(fc_b, np.float32).reshape(1, 1)
    return out
